# revision 1
# baseline (speedup 1.0000x reference)
"""Tensor-parallel GQA attention (CustomLlamaAttention) on 8 TRN2 NeuronCores.

Sharding: heads.  Core c owns Q heads 4c..4c+3 and KV head c.
  - Wq/Wkv output dims sharded; attention fully head-local per core.
  - Output projection sharded over Wo *rows* (output dim): each core computes
    out[:, 512c:512c+512] after an AllGather of the per-core attention
    outputs (transposed layout [hd, s]) -- cheaper than the all-reduce
    variant (4MB gather vs 32MB reduce).

Per-core dataflow (f32 storage, float32r matmuls at full PE rate):
  hidT [4096,2048] -> Q/KV projections -> qT/kT [hd,s] + partial RoPE,
  vT -> PE-transpose -> v [s,hd];  attention with S^T [sk,sq] layout:
  exp on ACT (no max subtraction: scores are O(10), fp32 exp is safe),
  softmax denominators via ones-column matmul, normalization via
  PE-broadcast of reciprocal sums; AllGather of attnT [512,2048] ->
  [4096,2048]; output projection -> out [2048,512] natural layout.

Matmul/DMA instructions can carry only one semaphore wait on this
toolchain (single EVENTS slot in the ISA); waitfix splits excess waits
onto sequencer NOPs.
"""

import sys

sys.path.insert(0, "/opt/trn_rl_repo")

import numpy as np

import concourse.bass as bass
import concourse.mybir as mybir
import concourse.tile as tile
from concourse.bass_utils import run_bass_kernel_spmd

# ---- problem constants (hardcoded per contract) ----
B, S, H = 1, 2048, 4096
NH, NKV, HD = 32, 8, 128
ROT = 64
BASE = 10000.0
N_CORES = 8
NH_LOC = NH // N_CORES  # 4 q heads per core
QD = NH_LOC * HD  # 512 local q dims
KVD = 2 * HD  # 256 local kv dims
CH = 512  # seq chunk (psum bank width in f32)
NCH = S // CH  # 4
ND = H // 128  # 32 contraction tiles
NT = S // 128  # 16 sk tiles
SCALE = 1.0 / float(np.sqrt(HD))

F32 = mybir.dt.float32
F32R = mybir.dt.float32r
BF16 = mybir.dt.bfloat16
AF = mybir.ActivationFunctionType
ALU = mybir.AluOpType


# --------------------------------------------------------------------------
# waitfix: split >1 semaphore waits per instruction onto sequencer NOPs
# --------------------------------------------------------------------------
def _split_waits(nc, max_waits=1):
    from concourse import bass_isa

    isa = nc.isa
    op = isa.Opcode.NEURON_ISA_TPB_OPCODE_NOP
    n_fixed = 0
    for f in nc.m.functions:
        for blk in f.blocks:
            il = blk.instructions
            fixes = []
            for i, inst in enumerate(il):
                si = inst.sync_info
                if si is None or len(si.on_wait) <= max_waits:
                    continue
                fixes.append((i, inst))
            for i, inst in reversed(fixes):
                si = inst.sync_info
                waits = list(si.on_wait)
                keep = waits[-max_waits:]
                nops = []
                for w in waits[:-max_waits]:
                    instr, fixups = bass_isa.isa_struct(isa, op, {})
                    nop = mybir.InstISA(
                        name=nc.get_next_instruction_name(),
                        isa_opcode=op.value,
                        engine=inst.engine,
                        instr=instr,
                        op_name="NOP",
                        ins=[],
                        outs=[],
                        ant_dict={},
                        verify=True,
                        ant_isa_is_sequencer_only=True,
                        ant_sbuf_fixups=fixups or None,
                    )
                    nop.sync_info = mybir.SyncInfo(on_wait=[w], on_update=[])
                    nops.append(nop)
                inst.sync_info = mybir.SyncInfo(on_wait=keep, on_update=si.on_update)
                for j, nop in enumerate(nops):
                    il.insert(i + j, nop)
                n_fixed += 1
    return n_fixed


# --------------------------------------------------------------------------
# kernel builder (SPMD program, same for all 8 cores)
# --------------------------------------------------------------------------
def build(causal: bool, skip_collective: bool = False, skip_waitfix: bool = False) -> bass.Bass:
    nc = bass.Bass()

    hidT = nc.declare_dram_parameter("hidT", [H, S], BF16, isOutput=False)
    wq_t = nc.declare_dram_parameter("wq_t", [H, QD], BF16, isOutput=False)
    wkv_t = nc.declare_dram_parameter("wkv_t", [H, KVD], BF16, isOutput=False)
    wo_t = nc.declare_dram_parameter("wo_t", [H, QD], BF16, isOutput=False)
    bq = nc.declare_dram_parameter("bq", [128, NH_LOC], F32, isOutput=False)
    bkv = nc.declare_dram_parameter("bkv", [128, 2], F32, isOutput=False)
    bo = nc.declare_dram_parameter("bo", [1, QD], F32R, isOutput=False)
    cosT = nc.declare_dram_parameter("cosT", [ROT, S], mybir.dt.bfloat16, isOutput=False)
    sinT = nc.declare_dram_parameter("sinT", [ROT, S], mybir.dt.bfloat16, isOutput=False)
    ones_col = nc.declare_dram_parameter("ones_col", [128, 1], F32R, isOutput=False)
    ones_row = nc.declare_dram_parameter("ones_row", [1, 128], F32R, isOutput=False)
    ident = nc.declare_dram_parameter("ident", [128, 128], F32R, isOutput=False)
    if causal:
        # 4 diagonal mask strips: strip j is the [sk 128, sq 512] transposed
        # mask block whose diagonal offset is 128*j
        maskT = nc.declare_dram_parameter("maskT", [4, 128, CH], F32, isOutput=False)
    else:
        maskT = nc.declare_dram_parameter("maskT", [S, S], F32, isOutput=False)
    out = nc.declare_dram_parameter("out", [S, QD], F32, isOutput=True)

    ag_in = nc.dram_tensor("ag_in", [QD, S], BF16)
    ag_out = nc.dram_tensor("ag_out", [H, S], BF16, addr_space="Shared")

    with tile.TileContext(nc) as tc:
        with tc.tile_pool(name="consts", bufs=1) as consts:
            # tiny consts up front; the big tables are DMA'd after chunk 0's
            # first d-group so they don't delay the first matmuls
            ones_col_t = consts.tile([128, 1], F32R)
            nc.sync.dma_start(out=ones_col_t, in_=ones_col[:, :])
            ones_row_t = consts.tile([1, 128], F32R)
            nc.sync.dma_start(out=ones_row_t, in_=ones_row[:, :])
            bq_t = consts.tile([128, NH_LOC], F32)
            nc.sync.dma_start(out=bq_t, in_=bq[:, :])
            bkv_t = consts.tile([128, 2], F32)
            nc.sync.dma_start(out=bkv_t, in_=bkv[:, :])
            ident_t = consts.tile([128, 128], F32R)
            cos_t = consts.tile([ROT, S], mybir.dt.bfloat16)
            sin_t = consts.tile([ROT, S], mybir.dt.bfloat16)
            bo_t = consts.tile([1, QD], F32R)
            mask_t = consts.tile([128, 4, CH], F32, name="mask_t") if causal else None

            def _load_big_consts():
                nc.sync.dma_start(out=cos_t, in_=cosT[:, :])
                nc.sync.dma_start(out=sin_t, in_=sinT[:, :])
                nc.sync.dma_start(out=ident_t, in_=ident[:, :])
                nc.sync.dma_start(out=bo_t, in_=bo[:, :])
                if causal:
                    nc.sync.dma_start(
                        out=mask_t, in_=maskT.rearrange("j p m -> p j m")
                    )

            with tc.tile_pool(name="qkv", bufs=1) as qkv:
                qT = qkv.tile([128, NH_LOC, S], F32R)  # [hd, head, sq]
                kT = qkv.tile([128, S], F32R)  # [hd, sk]
                vN = qkv.tile([128, NT, HD], F32R)  # [sk%128, sk tile, hd]

                # ---------------- phase A: projections -------------------
                with (
                    tc.tile_pool(name="wqkv", bufs=1) as wpool,
                    tc.tile_pool(name="hid", bufs=8) as hidp,
                    tc.tile_pool(name="ppsum", bufs=1, space="PSUM") as ppsum,
                    tc.tile_pool(name="ropetmp", bufs=2) as ropetmp,
                    tc.tile_pool(name="vtmp", bufs=1) as vtmp,
                    tc.tile_pool(name="tppsum", bufs=2, space="PSUM") as tppsum,
                ):
                    # weight tiles are loaded per-d, interleaved with chunk 0's
                    # hid tiles (inside the loop below) so the first matmuls
                    # start after ~2 small DMAs instead of the full 12MB
                    wq_tile = wpool.tile([128, ND, QD], BF16)
                    wkv_tile = wpool.tile([128, ND, KVD], BF16)
                    vT = vtmp.tile([128, S], F32R)  # [hd, sk], pre-transpose

                    for ch in range(NCH):
                        cs = slice(ch * CH, (ch + 1) * CH)
                        psums = [ppsum.tile([128, CH], F32, tag=f"pp{m}", name=f"pp{m}") for m in range(6)]
                        DG = 8
                        for grp in range(ND // DG):
                            if ch == 0 and grp == 1:
                                _load_big_consts()
                            hts = []
                            for dl in range(DG):
                                d = grp * DG + dl
                                ht = hidp.tile([128, CH], BF16, tag="hid")
                                nc.sync.dma_start(
                                    out=ht, in_=hidT[d * 128 : (d + 1) * 128, cs]
                                )
                                hts.append(ht)
                                if ch == 0:
                                    nc.sync.dma_start(
                                        out=wq_tile[:, d, :],
                                        in_=wq_t[d * 128 : (d + 1) * 128, :],
                                    )
                                    nc.sync.dma_start(
                                        out=wkv_tile[:, d, :],
                                        in_=wkv_t[d * 128 : (d + 1) * 128, :],
                                    )
                            for m in range(6):
                                if m < NH_LOC:
                                    w_ap = wq_tile[:, :, m * 128 : (m + 1) * 128]
                                else:
                                    mm = m - NH_LOC
                                    w_ap = wkv_tile[:, :, mm * 128 : (mm + 1) * 128]
                                for dl in range(DG):
                                    d = grp * DG + dl
                                    nc.tensor.matmul(
                                        psums[m][:, :],
                                        w_ap[:, d, :],
                                        hts[dl][:, :],
                                        start=(d == 0),
                                        stop=(d == ND - 1),
                                        skip_group_check=True,
                                    )
                        # stores: q heads with rope, k with rope, v plain
                        for m in range(NH_LOC):
                            _rope_store(
                                nc, ropetmp, qT[:, m, cs], psums[m],
                                bq_t[:, m : m + 1], cos_t[:, cs], sin_t[:, cs],
                            )
                        _rope_store(
                            nc, ropetmp, kT[:, cs], psums[4],
                            bkv_t[:, 0:1], cos_t[:, cs], sin_t[:, cs],
                        )
                        nc.vector.tensor_scalar_add(vT[:, cs], psums[5], bkv_t[:, 1:2])

                        # transpose this chunk's v: vT [hd, sk] -> vN [sk, hd]
                        for tl in range(CH // 128):
                            t = ch * (CH // 128) + tl
                            pst = tppsum.tile([128, 128], F32R)
                            nc.tensor.transpose(
                                pst[:, :], vT[:, t * 128 : (t + 1) * 128], ident_t[:, :]
                            )
                            nc.vector.tensor_copy(vN[:, t, :], pst[:, :])

                # phase B + C SBUF pools open together: the wo_t load streams
                # during attention, hiding the 8MB transfer
                with (
                    tc.tile_pool(name="wo", bufs=1) as wop,
                    tc.tile_pool(name="strip", bufs=6) as stripp,
                    tc.tile_pool(name="outp", bufs=3) as outp,
                ):
                    wo_tile = wop.tile([128, ND, QD], BF16)
                    for d in range(ND):
                        nc.sync.dma_start(
                            out=wo_tile[:, d, :],
                            in_=wo_t[d * 128 : (d + 1) * 128, :],
                        )

                    # ---------------- phase B: attention ---------------------
                    with (
                        tc.tile_pool(name="spsum", bufs=3, space="PSUM") as spsum,
                        tc.tile_pool(name="opsum", bufs=2, space="PSUM") as opsum,
                        tc.tile_pool(name="smpsum", bufs=2, space="PSUM") as smpsum,
                        tc.tile_pool(name="bcpsum", bufs=1, space="PSUM") as bcpsum,
                        tc.tile_pool(name="ptile", bufs=4) as ptile,
                        tc.tile_pool(name="btmp", bufs=3) as btmp,
                    ):
                        # start with chunk 3: its first 12 sk-tiles need no
                        # mask (ACT-only exp), giving PE runway while chunk
                        # 3's rope stores drain on DVE
                        ch_order = [3, 2, 1, 0] if causal else list(range(NCH))
                        for ch in ch_order:
                            cs = slice(ch * CH, (ch + 1) * CH)
                            nt = 4 * (ch + 1) if causal else NT
                            for h in range(NH_LOC):
                                ps_o = opsum.tile([128, CH], F32, tag="o")
                                ps_sum = smpsum.tile([1, CH], F32, tag="s")
                                for t in range(nt):
                                    diag_j = t - 4 * ch
                                    # causal: diag strip j only needs sq >= 128j.
                                    # Cap at 256 so the f32r moving dim stays
                                    # >=256 (full PE rate); the mask covers the
                                    # extra columns.
                                    sq0 = (
                                        min(128 * diag_j, 256)
                                        if (causal and diag_j > 0)
                                        else 0
                                    )
                                    qs_ap = qT[:, h, ch * CH + sq0 : (ch + 1) * CH]
                                    ps_s = spsum.tile([128, CH], F32, tag="sc")
                                    nc.tensor.matmul(
                                        ps_s[:, sq0:],
                                        kT[:, t * 128 : (t + 1) * 128],
                                        qs_ap,
                                        start=True,
                                        stop=True,
                                    )
                                    p = ptile.tile([128, CH], F32R, tag="p")
                                    if causal and 0 <= diag_j < 4:
                                        sb_s = btmp.tile([128, CH], F32, tag="sb")
                                        nc.vector.scalar_tensor_tensor(
                                            sb_s[:, sq0:], ps_s[:, sq0:], SCALE,
                                            mask_t[:, diag_j, sq0:],
                                            op0=ALU.mult, op1=ALU.add,
                                        )
                                        nc.scalar.activation(
                                            p[:, sq0:], sb_s[:, sq0:], AF.Exp
                                        )
                                    elif not causal:
                                        mt = btmp.tile([128, CH], F32, tag="mt", bufs=2)
                                        nc.sync.dma_start(
                                            out=mt,
                                            in_=maskT[t * 128 : (t + 1) * 128, cs],
                                        )
                                        sb_s = btmp.tile([128, CH], F32, tag="sb")
                                        nc.vector.scalar_tensor_tensor(
                                            sb_s[:, :], ps_s[:, :], SCALE,
                                            mt[:, :],
                                            op0=ALU.mult, op1=ALU.add,
                                        )
                                        nc.scalar.activation(
                                            p[:, :], sb_s[:, :], AF.Exp
                                        )
                                    else:
                                        nc.scalar.activation(
                                            p[:, :], ps_s[:, :], AF.Exp, scale=SCALE
                                        )
                                    nc.tensor.matmul(
                                        ps_o[:, sq0:], vN[:, t, :], p[:, sq0:],
                                        start=(t == 0), stop=(t == nt - 1),
                                        skip_group_check=True,
                                    )
                                    nc.tensor.matmul(
                                        ps_sum[:, sq0:], ones_col_t[:, :], p[:, sq0:],
                                        start=(t == 0), stop=(t == nt - 1),
                                        skip_group_check=True,
                                    )
                                # normalize: out *= 1/sums (broadcast over parts)
                                rec = btmp.tile([1, CH], F32R, tag="rec")
                                with nc.allow_low_precision(reason="f32r recip"):
                                    nc.vector.reciprocal(rec[:, :], ps_sum[:, :])
                                ps_bc = bcpsum.tile([128, CH], F32, tag="bc")
                                nc.tensor.matmul(
                                    ps_bc[:, :], ones_row_t[:, :], rec[:, :],
                                    start=True, stop=True,
                                )
                                bc_sb = btmp.tile([128, CH], F32, tag="bcs")
                                nc.vector.tensor_copy(bc_sb[:, :], ps_bc[:, :])
                                an = btmp.tile([128, CH], BF16, tag="an")
                                nc.vector.tensor_mul(an[:, :], ps_o[:, :], bc_sb[:, :])
                                nc.sync.dma_start(
                                    out=ag_in[h * 128 : (h + 1) * 128, cs],
                                    in_=an[:, :],
                                )

                    # ---------------- all-gather -------------------------
                    if not skip_collective:
                        nc.gpsimd.collective_compute(
                            "AllGather",
                            ALU.bypass,
                            ins=[ag_in[:, :]],
                            outs=[ag_out[:, :]],
                            replica_groups=[list(range(N_CORES))],
                        )

                    # ---------------- phase C: output projection ----------
                    with tc.tile_pool(name="copsum", bufs=2, space="PSUM") as copsum:
                        ps_bo = copsum.tile([128, QD], F32, tag="co0", name="psbo")
                        nc.tensor.matmul(
                            ps_bo[:, :], ones_row_t[:, :], bo_t[:, :],
                            start=True, stop=True,
                        )
                        bo_bc = outp.tile([128, QD], F32, tag="bo")
                        nc.vector.tensor_copy(bo_bc[:, :], ps_bo[:, :])

                        for sqb in range(NCH):
                            ps_outs = [
                                copsum.tile(
                                    [128, QD], F32, tag=f"co{j}", name=f"co{j}"
                                )
                                for j in range(4)
                            ]
                            for d in range(ND):
                                strip = stripp.tile([128, CH], BF16, tag="strip")
                                nc.sync.dma_start(
                                    out=strip,
                                    in_=ag_out[
                                        d * 128 : (d + 1) * 128,
                                        sqb * CH : (sqb + 1) * CH,
                                    ],
                                )
                                for j in range(4):
                                    nc.tensor.matmul(
                                        ps_outs[j][:, :],
                                        strip[:, j * 128 : (j + 1) * 128],
                                        wo_tile[:, d, :],
                                        start=(d == 0),
                                        stop=(d == ND - 1),
                                        skip_group_check=True,
                                    )
                            for j in range(4):
                                ot = outp.tile([128, QD], F32, tag="ot")
                                nc.vector.tensor_add(
                                    ot[:, :], ps_outs[j][:, :], bo_bc[:, :]
                                )
                                nc.sync.dma_start(
                                    out=out[
                                        sqb * CH + j * 128 : sqb * CH + (j + 1) * 128,
                                        :,
                                    ],
                                    in_=ot[:, :],
                                )

    if not skip_waitfix:
        _split_waits(nc)
    return nc


def _rope_store(nc, tmp, dst, ps, bias, cos_s, sin_s):
    """dst[hd, s] (f32r) = rope(ps + bias) for rows 0:64; pass rows 64:128.

    cos_s [64, CH]: cos table duplicated over both 32-row halves.
    sin_s [64, CH]: sign-folded sin: rows 0:32 = -sin, rows 32:64 = +sin, so
      new[0:64] = x[0:64]*cos_s + rot32(x[0:64])*sin_s
    with rot32 = swap of the two 32-row halves (done via SBUF->SBUF DMA,
    since compute engines cannot cross partitions).
    """
    R2 = ROT // 2
    # bias-added copy of the whole tile straight into the destination
    nc.vector.tensor_scalar_add(dst[:, :], ps[:, :], bias[:, :])
    # partition-rotated rope rows
    qsh = tmp.tile([ROT, CH], F32R, tag="r1")
    nc.sync.dma_start(out=qsh[0:R2, :], in_=dst[R2:ROT, :])
    nc.sync.dma_start(out=qsh[R2:ROT, :], in_=dst[0:R2, :])
    t1 = tmp.tile([ROT, CH], F32, tag="r2")
    nc.vector.tensor_mul(t1[:, :], dst[0:ROT, :], cos_s[:, :])
    t2 = tmp.tile([ROT, CH], F32, tag="r3")
    nc.vector.tensor_mul(t2[:, :], qsh[:, :], sin_s[:, :])
    nc.vector.tensor_add(dst[0:ROT, :], t1[:, :], t2[:, :])


# --------------------------------------------------------------------------
# host side: shard, run, gather
# --------------------------------------------------------------------------
_NC_CACHE = {}


def _get_nc(causal: bool) -> bass.Bass:
    if causal not in _NC_CACHE:
        _NC_CACHE[causal] = build(causal)
    return _NC_CACHE[causal]


def _rope_tables():
    inv_freq = 1.0 / (BASE ** (np.arange(0, ROT, 2, dtype=np.float64) / ROT))
    t = np.arange(S, dtype=np.float64)
    freqs = np.outer(t, inv_freq)  # [S, 32]
    import ml_dtypes

    cos32 = np.cos(freqs).T.astype(np.float32)  # [32, S]
    sin32 = np.sin(freqs).T.astype(np.float32)
    cosT = np.concatenate([cos32, cos32], axis=0).astype(ml_dtypes.bfloat16)
    sinT = np.concatenate([-sin32, sin32], axis=0).astype(ml_dtypes.bfloat16)
    return cosT, sinT


def _check_causal(mask):
    """mask: [1,1,S,S]. True if it is exactly a causal additive mask."""
    m = mask[0, 0]
    if not (m[np.tril_indices(S)] == 0.0).all():
        return False
    iu = np.triu_indices(S, k=1)
    vals = m[iu]
    return bool((vals <= -1e30).all()) and bool((vals == vals[0]).all())


def _make_in_maps(inputs, causal):
    import ml_dtypes
    hidden = np.asarray(inputs["hidden_states"], dtype=np.float32)
    mask = np.asarray(inputs["attention_mask"], dtype=np.float32)
    Wq = np.asarray(inputs["Wq"], dtype=np.float32)
    bq = np.asarray(inputs["bq"], dtype=np.float32)
    Wkv = np.asarray(inputs["Wkv"], dtype=np.float32)
    bkv = np.asarray(inputs["bkv"], dtype=np.float32)
    Wo = np.asarray(inputs["Wo"], dtype=np.float32)
    bo = np.asarray(inputs["bo"], dtype=np.float32)

    hidT_bf = np.ascontiguousarray(hidden[0].T).astype(ml_dtypes.bfloat16)  # [H, S]
    cosT, sinT = _rope_tables()
    ones_col = np.ones((128, 1), np.float32)
    ones_row = np.ones((1, 128), np.float32)
    ident = np.eye(128, dtype=np.float32)

    if causal:
        # diagonal strips from the actual mask (chunk 0 is representative —
        # _check_causal guarantees the pattern is uniform along the diagonal)
        maskT = np.stack(
            [np.ascontiguousarray(mask[0, 0, 0:CH, 128 * j : 128 * j + 128].T)
             for j in range(4)]
        )  # [4, 128, CH]
    else:
        maskT = np.ascontiguousarray(mask[0, 0].T)  # [S, S]

    in_maps = []
    for c in range(N_CORES):
        qs = slice(c * QD, (c + 1) * QD)
        kvs = slice(c * KVD, (c + 1) * KVD)
        in_maps.append(
            {
                "hidT": hidT_bf,
                "wq_t": np.ascontiguousarray(Wq[qs, :].T).astype(ml_dtypes.bfloat16),
                "wkv_t": np.ascontiguousarray(Wkv[kvs, :].T).astype(ml_dtypes.bfloat16),
                "wo_t": np.ascontiguousarray(Wo[qs, :].T).astype(ml_dtypes.bfloat16),
                "bq": np.ascontiguousarray(bq[qs].reshape(NH_LOC, 128).T),
                "bkv": np.ascontiguousarray(
                    bkv[kvs].reshape(2, 128).T
                ),
                "bo": bo[qs].reshape(1, QD),
                "cosT": cosT,
                "sinT": sinT,
                "ones_col": ones_col,
                "ones_row": ones_row,
                "ident": ident,
                "maskT": maskT,
            }
        )
    return in_maps


def kernel(**inputs) -> np.ndarray:
    causal = _check_causal(np.asarray(inputs["attention_mask"], dtype=np.float32))
    nc = _get_nc(causal)
    in_maps = _make_in_maps(inputs, causal)
    res = run_bass_kernel_spmd(nc, in_maps, list(range(N_CORES)))
    outs = [res.results[c]["out"] for c in range(N_CORES)]  # each [S, QD]
    full = np.concatenate(outs, axis=1)  # [S, H]
    return full.reshape(B, S, H)



# revision 19
# speedup vs baseline: 1.0436x; 1.0436x over previous
"""Tensor-parallel GQA attention (CustomLlamaAttention) on 8 TRN2 NeuronCores.

Sharding: heads.  Core c owns Q heads 4c..4c+3 and KV head c.
  - Wq/Wkv output dims sharded; attention fully head-local per core.
  - Output projection sharded over Wo *rows* (output dim): each core computes
    out[:, 512c:512c+512] after an AllGather of the per-core attention
    outputs (transposed layout [hd, s]) -- cheaper than the all-reduce
    variant (4MB gather vs 32MB reduce).

Per-core dataflow (f32 storage, float32r matmuls at full PE rate):
  hidT [4096,2048] -> Q/KV projections -> qT/kT [hd,s] + partial RoPE,
  vT -> PE-transpose -> v [s,hd];  attention with S^T [sk,sq] layout:
  exp on ACT (no max subtraction: scores are O(10), fp32 exp is safe),
  softmax denominators via ones-column matmul, normalization via
  PE-broadcast of reciprocal sums; AllGather of attnT [512,2048] ->
  [4096,2048]; output projection -> out [2048,512] natural layout.

Matmul/DMA instructions can carry only one semaphore wait on this
toolchain (single EVENTS slot in the ISA); waitfix splits excess waits
onto sequencer NOPs.
"""

import sys

sys.path.insert(0, "/opt/trn_rl_repo")

import numpy as np

import concourse.bass as bass
import concourse.mybir as mybir
import concourse.tile as tile
from concourse.bass_utils import run_bass_kernel_spmd

# ---- problem constants (hardcoded per contract) ----
B, S, H = 1, 2048, 4096
NH, NKV, HD = 32, 8, 128
ROT = 64
BASE = 10000.0
N_CORES = 8
NH_LOC = NH // N_CORES  # 4 q heads per core
QD = NH_LOC * HD  # 512 local q dims
KVD = 2 * HD  # 256 local kv dims
CH = 512  # seq chunk (psum bank width in f32)
NCH = S // CH  # 4
ND = H // 128  # 32 contraction tiles
NT = S // 128  # 16 sk tiles
SCALE = 1.0 / float(np.sqrt(HD))

F32 = mybir.dt.float32
F32R = mybir.dt.float32r
BF16 = mybir.dt.bfloat16
AF = mybir.ActivationFunctionType
ALU = mybir.AluOpType


# --------------------------------------------------------------------------
# waitfix: split >1 semaphore waits per instruction onto sequencer NOPs
# --------------------------------------------------------------------------
def _split_waits(nc, max_waits=1):
    from concourse import bass_isa

    isa = nc.isa
    op = isa.Opcode.NEURON_ISA_TPB_OPCODE_NOP
    n_fixed = 0
    for f in nc.m.functions:
        for blk in f.blocks:
            il = blk.instructions
            fixes = []
            for i, inst in enumerate(il):
                si = inst.sync_info
                if si is None or len(si.on_wait) <= max_waits:
                    continue
                fixes.append((i, inst))
            for i, inst in reversed(fixes):
                si = inst.sync_info
                waits = list(si.on_wait)
                keep = waits[-max_waits:]
                nops = []
                for w in waits[:-max_waits]:
                    instr, fixups = bass_isa.isa_struct(isa, op, {})
                    nop = mybir.InstISA(
                        name=nc.get_next_instruction_name(),
                        isa_opcode=op.value,
                        engine=inst.engine,
                        instr=instr,
                        op_name="NOP",
                        ins=[],
                        outs=[],
                        ant_dict={},
                        verify=True,
                        ant_isa_is_sequencer_only=True,
                        ant_sbuf_fixups=fixups or None,
                    )
                    nop.sync_info = mybir.SyncInfo(on_wait=[w], on_update=[])
                    nops.append(nop)
                inst.sync_info = mybir.SyncInfo(on_wait=keep, on_update=si.on_update)
                for j, nop in enumerate(nops):
                    il.insert(i + j, nop)
                n_fixed += 1
    return n_fixed


# --------------------------------------------------------------------------
# kernel builder (SPMD program, same for all 8 cores)
# --------------------------------------------------------------------------
def build(causal: bool, skip_collective: bool = False, skip_waitfix: bool = False) -> bass.Bass:
    nc = bass.Bass()

    hidT = nc.declare_dram_parameter("hidT", [H, S], BF16, isOutput=False)
    wq_t = nc.declare_dram_parameter("wq_t", [H, QD], BF16, isOutput=False)
    wkv_t = nc.declare_dram_parameter("wkv_t", [H, KVD], BF16, isOutput=False)
    wo_t = nc.declare_dram_parameter("wo_t", [H, QD], BF16, isOutput=False)
    bq = nc.declare_dram_parameter("bq", [128, NH_LOC], F32, isOutput=False)
    bkv = nc.declare_dram_parameter("bkv", [128, 2], F32, isOutput=False)
    bo = nc.declare_dram_parameter("bo", [1, QD], F32R, isOutput=False)
    cosT = nc.declare_dram_parameter("cosT", [ROT, S], mybir.dt.bfloat16, isOutput=False)
    sinT = nc.declare_dram_parameter("sinT", [ROT, S], mybir.dt.bfloat16, isOutput=False)
    ones_col = nc.declare_dram_parameter("ones_col", [128, 1], F32R, isOutput=False)
    ones_row = nc.declare_dram_parameter("ones_row", [1, 128], F32R, isOutput=False)
    ident = nc.declare_dram_parameter("ident", [128, 128], F32R, isOutput=False)
    if causal:
        # 4 diagonal mask strips: strip j is the [sk 128, sq 512] transposed
        # mask block whose diagonal offset is 128*j (bf16: 0 / -3.39e38 exact)
        maskT = nc.declare_dram_parameter("maskT", [4, 128, CH], BF16, isOutput=False)
    else:
        maskT = nc.declare_dram_parameter("maskT", [S, S], F32, isOutput=False)
    out = nc.declare_dram_parameter("out", [S, QD], F32, isOutput=True)

    # per-seq-chunk collective tensors: AG(ch) fires as soon as attention
    # for chunk ch is done on all cores, overlapping later attention chunks
    ag_in = [nc.dram_tensor(f"ag_in{ch}", [QD, CH], BF16) for ch in range(NCH)]
    ag_out = [
        nc.dram_tensor(f"ag_out{ch}", [H, CH], BF16, addr_space="Shared")
        for ch in range(NCH)
    ]

    with tile.TileContext(nc) as tc:
        with tc.tile_pool(name="consts", bufs=1) as consts:
            # tiny consts up front; the big tables are DMA'd after chunk 0's
            # first d-group so they don't delay the first matmuls
            ones_col_t = consts.tile([128, 1], F32R)
            nc.sync.dma_start(out=ones_col_t, in_=ones_col[:, :])
            ones_row_t = consts.tile([1, 128], F32R)
            nc.sync.dma_start(out=ones_row_t, in_=ones_row[:, :])
            bq_t = consts.tile([128, NH_LOC], F32)
            nc.sync.dma_start(out=bq_t, in_=bq[:, :])
            bkv_t = consts.tile([128, 2], F32)
            nc.sync.dma_start(out=bkv_t, in_=bkv[:, :])
            ident_t = consts.tile([128, 128], F32R)
            cos_t = consts.tile([ROT, S], mybir.dt.bfloat16)
            sin_t = consts.tile([ROT, S], mybir.dt.bfloat16)
            bo_t = consts.tile([1, QD], F32R)
            mask_t = consts.tile([128, 4, CH], BF16, name="mask_t") if causal else None
            # preload the ACT exp table (~1.3us) far ahead of the first real
            # exp so it is off the phase-A -> attention critical path
            warm_t = consts.tile([1, 1], F32)
            nc.scalar.activation(warm_t[:, :], ones_col_t[0:1, 0:1], AF.Exp)

            # chunk 0 is DMA-throughput-bound (hid + all weights), so only
            # the tables chunk 0's own stores need (cos/sin/ident, ~2us) load
            # there; mask/bo wait for chunk 1 where DMA has ~30us of slack
            def _load_tables():
                nc.sync.dma_start(out=cos_t, in_=cosT[:, :])
                nc.sync.dma_start(out=sin_t, in_=sinT[:, :])
                nc.sync.dma_start(out=ident_t, in_=ident[:, :])

            def _load_mask():
                nc.sync.dma_start(out=bo_t, in_=bo[:, :])
                if causal:
                    nc.sync.dma_start(
                        out=mask_t, in_=maskT.rearrange("j p m -> p j m")
                    )

            # ropetmp/vtmp stay open for the whole kernel: their SBUF ranges
            # must NOT be recycled into attention's pools, or attention's
            # first tiles inherit a WAR on the last rope chain and the
            # whole A->B seam serializes behind it
            with (
                tc.tile_pool(name="qkv", bufs=1) as qkv,
                tc.tile_pool(name="ropetmp", bufs=2) as ropetmp,
                tc.tile_pool(name="vtmp", bufs=1) as vtmp,
            ):
                qT = qkv.tile([128, NH_LOC, S], F32R)  # [hd, head, sq]
                kT = qkv.tile([128, S], F32R)  # [hd, sk]
                vN = qkv.tile([128, NT, HD], F32R)  # [sk%128, sk tile, hd]

                # ---------------- phase A: projections -------------------
                with (
                    tc.tile_pool(name="wqkv", bufs=1) as wpool,
                    tc.tile_pool(name="hid", bufs=16) as hidp,
                    tc.tile_pool(name="ppsum", bufs=1, space="PSUM") as ppsum,
                    tc.tile_pool(name="tppsum", bufs=2, space="PSUM") as tppsum,
                ):
                    # weight tiles are loaded per-d, interleaved with chunk 0's
                    # hid tiles (inside the loop below) so the first matmuls
                    # start after ~2 small DMAs instead of the full 12MB
                    wq_tile = wpool.tile([128, ND, QD], BF16)
                    wkv_tile = wpool.tile([128, ND, KVD], BF16)
                    vT = vtmp.tile([128, S], F32R)  # [hd, sk], pre-transpose

                    for ch in range(NCH):
                        cs = slice(ch * CH, (ch + 1) * CH)
                        psums = [ppsum.tile([128, CH], F32, tag=f"pp{m}", name=f"pp{m}") for m in range(6)]
                        DG = 8
                        NGRP = ND // DG

                        for grp in range(NGRP):
                            hts = []
                            for dl in range(DG):
                                d = grp * DG + dl
                                ht = hidp.tile([128, CH], BF16, tag="hid")
                                nc.sync.dma_start(
                                    out=ht, in_=hidT[d * 128 : (d + 1) * 128, cs]
                                )
                                hts.append(ht)
                                if ch == 0:
                                    nc.sync.dma_start(
                                        out=wq_tile[:, d, :],
                                        in_=wq_t[d * 128 : (d + 1) * 128, :],
                                    )
                                    nc.sync.dma_start(
                                        out=wkv_tile[:, d, :],
                                        in_=wkv_t[d * 128 : (d + 1) * 128, :],
                                    )
                            # const loads queue AFTER the group's hid/weight
                            # DMAs so they never delay the tiles PE is about
                            # to consume; still in time for chunk-0's rope
                            # stores (cos/sin/ident) and attention (mask)
                            if ch == 0 and grp == 2:
                                _load_tables()
                            elif ch == 1 and grp == 1:
                                _load_mask()
                            # d-major: each d consumes its (ht, wq, wkv) DMAs
                            # right after they land, so PE paces the DMA
                            # stream instead of stalling a whole group on it
                            # (chunk 0 is DMA-fed at ~90% of PE rate)
                            for dl in range(DG):
                                d = grp * DG + dl
                                for m in range(6):
                                    if m < NH_LOC:
                                        w_ap = wq_tile[:, :, m * 128 : (m + 1) * 128]
                                    else:
                                        mm = m - NH_LOC
                                        w_ap = wkv_tile[:, :, mm * 128 : (mm + 1) * 128]
                                    nc.tensor.matmul(
                                        psums[m][:, :],
                                        w_ap[:, d, :],
                                        hts[dl][:, :],
                                        start=(d == 0),
                                        stop=(d == ND - 1),
                                        skip_group_check=True,
                                    )
                        # stores: q0 first (frees pp0 for the next chunk's
                        # first matmul), then k (attention needs kT), q1-3, v
                        _rope_store(
                            nc, ropetmp, qT[:, 0, cs], psums[0],
                            bq_t[:, 0:1], cos_t[:, cs], sin_t[:, cs],
                        )
                        _rope_store(
                            nc, ropetmp, kT[:, cs], psums[4],
                            bkv_t[:, 0:1], cos_t[:, cs], sin_t[:, cs],
                        )
                        for m in range(1, NH_LOC):
                            _rope_store(
                                nc, ropetmp, qT[:, m, cs], psums[m],
                                bq_t[:, m : m + 1], cos_t[:, cs], sin_t[:, cs],
                            )
                        nc.vector.tensor_scalar_add(vT[:, cs], psums[5], bkv_t[:, 1:2])

                        # transpose this chunk's v: vT [hd, sk] -> vN [sk, hd]
                        for tl in range(CH // 128):
                            t = ch * (CH // 128) + tl
                            pst = tppsum.tile([128, 128], F32R)
                            nc.tensor.transpose(
                                pst[:, :], vT[:, t * 128 : (t + 1) * 128], ident_t[:, :]
                            )
                            nc.vector.tensor_copy(vN[:, t, :], pst[:, :])

                # phase B + C SBUF pools open together: the wo_t load streams
                # during attention, hiding the 8MB transfer
                with (
                    tc.tile_pool(name="wo", bufs=1) as wop,
                    tc.tile_pool(name="strip", bufs=6) as stripp,
                    tc.tile_pool(name="outp", bufs=3) as outp,
                ):
                    wo_tile = wop.tile([128, ND, QD], BF16)
                    for d in range(ND):
                        nc.sync.dma_start(
                            out=wo_tile[:, d, :],
                            in_=wo_t[d * 128 : (d + 1) * 128, :],
                        )

                    # ---------------- phase B: attention ---------------------
                    with (
                        tc.tile_pool(name="spsum", bufs=3, space="PSUM") as spsum,
                        tc.tile_pool(name="opsum", bufs=2, space="PSUM") as opsum,
                        tc.tile_pool(name="smpsum", bufs=2, space="PSUM") as smpsum,
                        tc.tile_pool(name="bcpsum", bufs=1, space="PSUM") as bcpsum,
                        tc.tile_pool(name="ptile", bufs=4) as ptile,
                        tc.tile_pool(name="btmp", bufs=3) as btmp,
                    ):
                        # start with chunk 2: its q/k tiles have been ready
                        # since A's chunk-2 finished (~44us of slack) and 8
                        # of its 12 sk-tiles are mask-free, so attention
                        # opens at full rate while chunk 3's rope stores
                        # drain on DVE/Pool behind it
                        ch_order = [2, 3, 1, 0] if causal else list(range(NCH))
                        for ch in ch_order:
                            cs = slice(ch * CH, (ch + 1) * CH)
                            nt = 4 * (ch + 1) if causal else NT
                            for h in range(NH_LOC):
                                ps_o = opsum.tile([128, CH], F32, tag="o")
                                ps_sum = smpsum.tile([1, CH], F32, tag="s")
                                for t in range(nt):
                                    diag_j = t - 4 * ch
                                    # causal: diag strip j only needs sq >= 128j.
                                    # Cap at 256 so the f32r moving dim stays
                                    # >=256 (full PE rate); the mask covers the
                                    # extra columns.
                                    sq0 = (
                                        min(128 * diag_j, 256)
                                        if (causal and diag_j > 0)
                                        else 0
                                    )
                                    qs_ap = qT[:, h, ch * CH + sq0 : (ch + 1) * CH]
                                    ps_s = spsum.tile([128, CH], F32, tag="sc")
                                    nc.tensor.matmul(
                                        ps_s[:, sq0:],
                                        kT[:, t * 128 : (t + 1) * 128],
                                        qs_ap,
                                        start=True,
                                        stop=True,
                                    )
                                    p = ptile.tile([128, CH], F32R, tag="p")
                                    if causal and 0 <= diag_j < 4:
                                        # DVE, not Pool: GPSIMD cannot read
                                        # PSUM (ps_s) per the BIR verifier
                                        sb_s = btmp.tile([128, CH], F32, tag="sb")
                                        nc.vector.scalar_tensor_tensor(
                                            sb_s[:, sq0:], ps_s[:, sq0:], SCALE,
                                            mask_t[:, diag_j, sq0:],
                                            op0=ALU.mult, op1=ALU.add,
                                        )
                                        nc.scalar.activation(
                                            p[:, sq0:], sb_s[:, sq0:], AF.Exp
                                        )
                                    elif not causal:
                                        mt = btmp.tile([128, CH], F32, tag="mt", bufs=2)
                                        nc.sync.dma_start(
                                            out=mt,
                                            in_=maskT[t * 128 : (t + 1) * 128, cs],
                                        )
                                        sb_s = btmp.tile([128, CH], F32, tag="sb")
                                        nc.vector.scalar_tensor_tensor(
                                            sb_s[:, :], ps_s[:, :], SCALE,
                                            mt[:, :],
                                            op0=ALU.mult, op1=ALU.add,
                                        )
                                        nc.scalar.activation(
                                            p[:, :], sb_s[:, :], AF.Exp
                                        )
                                    else:
                                        nc.scalar.activation(
                                            p[:, :], ps_s[:, :], AF.Exp, scale=SCALE
                                        )
                                    nc.tensor.matmul(
                                        ps_o[:, sq0:], vN[:, t, :], p[:, sq0:],
                                        start=(t == 0), stop=(t == nt - 1),
                                        skip_group_check=True,
                                    )
                                    nc.tensor.matmul(
                                        ps_sum[:, sq0:], ones_col_t[:, :], p[:, sq0:],
                                        start=(t == 0), stop=(t == nt - 1),
                                        skip_group_check=True,
                                    )
                                # normalize: out *= 1/sums (broadcast over parts)
                                rec = btmp.tile([1, CH], F32R, tag="rec")
                                with nc.allow_low_precision(reason="f32r recip"):
                                    nc.vector.reciprocal(rec[:, :], ps_sum[:, :])
                                ps_bc = bcpsum.tile([128, CH], F32, tag="bc")
                                nc.tensor.matmul(
                                    ps_bc[:, :], ones_row_t[:, :], rec[:, :],
                                    start=True, stop=True,
                                )
                                bc_sb = btmp.tile([128, CH], F32, tag="bcs")
                                nc.vector.tensor_copy(bc_sb[:, :], ps_bc[:, :])
                                an = btmp.tile([128, CH], BF16, tag="an")
                                nc.vector.tensor_mul(an[:, :], ps_o[:, :], bc_sb[:, :])
                                nc.sync.dma_start(
                                    out=ag_in[ch][h * 128 : (h + 1) * 128, :],
                                    in_=an[:, :],
                                )
                            # chunked all-gather: fire as soon as this seq
                            # chunk's 4 heads are stored; overlaps the
                            # remaining attention chunks and phase C
                            if not skip_collective:
                                nc.gpsimd.collective_compute(
                                    "AllGather",
                                    ALU.bypass,
                                    ins=[ag_in[ch][:, :]],
                                    outs=[ag_out[ch][:, :]],
                                    replica_groups=[list(range(N_CORES))],
                                )

                    # ---------------- phase C: output projection ----------
                    with tc.tile_pool(name="copsum", bufs=2, space="PSUM") as copsum:
                        ps_bo = copsum.tile([128, QD], F32, tag="co0", name="psbo")
                        nc.tensor.matmul(
                            ps_bo[:, :], ones_row_t[:, :], bo_t[:, :],
                            start=True, stop=True,
                        )
                        bo_bc = outp.tile([128, QD], F32, tag="bo")
                        nc.vector.tensor_copy(bo_bc[:, :], ps_bo[:, :])

                        # same order as attention emitted its chunks: sqb's
                        # gather lands earliest for the chunk attention
                        # finished first, so phase C never waits on an AG
                        sq_order = [2, 3, 1, 0] if causal else list(range(NCH))
                        for sqb in sq_order:
                            ps_outs = [
                                copsum.tile(
                                    [128, QD], F32, tag=f"co{j}", name=f"co{j}"
                                )
                                for j in range(4)
                            ]
                            for d in range(ND):
                                strip = stripp.tile([128, CH], BF16, tag="strip")
                                nc.sync.dma_start(
                                    out=strip,
                                    in_=ag_out[sqb][d * 128 : (d + 1) * 128, :],
                                )
                                for j in range(4):
                                    nc.tensor.matmul(
                                        ps_outs[j][:, :],
                                        strip[:, j * 128 : (j + 1) * 128],
                                        wo_tile[:, d, :],
                                        start=(d == 0),
                                        stop=(d == ND - 1),
                                        skip_group_check=True,
                                    )
                            for j in range(4):
                                ot = outp.tile([128, QD], F32, tag="ot")
                                nc.vector.tensor_add(
                                    ot[:, :], ps_outs[j][:, :], bo_bc[:, :]
                                )
                                nc.sync.dma_start(
                                    out=out[
                                        sqb * CH + j * 128 : sqb * CH + (j + 1) * 128,
                                        :,
                                    ],
                                    in_=ot[:, :],
                                )

    if not skip_waitfix:
        _split_waits(nc)
    return nc


def _rope_store(nc, tmp, dst, ps, bias, cos_s, sin_s):
    """dst[hd, s] (f32r) = rope(ps + bias) for rows 0:64; pass rows 64:128.

    cos_s [64, CH]: cos table duplicated over both 32-row halves.
    sin_s [64, CH]: sign-folded sin: rows 0:32 = -sin, rows 32:64 = +sin, so
      new[0:64] = x[0:64]*cos_s + rot32(x[0:64])*sin_s
    with rot32 = swap of the two 32-row halves (done via SBUF->SBUF DMA,
    since compute engines cannot cross partitions).
    """
    R2 = ROT // 2
    # bias-added copy of the whole tile straight into the destination
    nc.vector.tensor_scalar_add(dst[:, :], ps[:, :], bias[:, :])
    # partition-rotated rope rows
    qsh = tmp.tile([ROT, CH], F32R, tag="r1")
    nc.sync.dma_start(out=qsh[0:R2, :], in_=dst[R2:ROT, :])
    nc.sync.dma_start(out=qsh[R2:ROT, :], in_=dst[0:R2, :])
    # the two muls run on the (otherwise idle) Pool engine so the DVE only
    # carries the add chain -- keeps rope off the chunk-boundary critical path
    t1 = tmp.tile([ROT, CH], F32, tag="r2")
    nc.gpsimd.tensor_mul(t1[:, :], dst[0:ROT, :], cos_s[:, :])
    t2 = tmp.tile([ROT, CH], F32, tag="r3")
    nc.gpsimd.tensor_mul(t2[:, :], qsh[:, :], sin_s[:, :])
    nc.vector.tensor_add(dst[0:ROT, :], t1[:, :], t2[:, :])


# --------------------------------------------------------------------------
# host side: shard, run, gather
# --------------------------------------------------------------------------
_NC_CACHE = {}


def _get_nc(causal: bool) -> bass.Bass:
    if causal not in _NC_CACHE:
        _NC_CACHE[causal] = build(causal)
    return _NC_CACHE[causal]


def _rope_tables():
    inv_freq = 1.0 / (BASE ** (np.arange(0, ROT, 2, dtype=np.float64) / ROT))
    t = np.arange(S, dtype=np.float64)
    freqs = np.outer(t, inv_freq)  # [S, 32]
    import ml_dtypes

    cos32 = np.cos(freqs).T.astype(np.float32)  # [32, S]
    sin32 = np.sin(freqs).T.astype(np.float32)
    cosT = np.concatenate([cos32, cos32], axis=0).astype(ml_dtypes.bfloat16)
    sinT = np.concatenate([-sin32, sin32], axis=0).astype(ml_dtypes.bfloat16)
    return cosT, sinT


def _check_causal(mask):
    """mask: [1,1,S,S]. True if it is exactly a causal additive mask."""
    m = mask[0, 0]
    if not (m[np.tril_indices(S)] == 0.0).all():
        return False
    iu = np.triu_indices(S, k=1)
    vals = m[iu]
    return bool((vals <= -1e30).all()) and bool((vals == vals[0]).all())


def _make_in_maps(inputs, causal):
    import ml_dtypes
    hidden = np.asarray(inputs["hidden_states"], dtype=np.float32)
    mask = np.asarray(inputs["attention_mask"], dtype=np.float32)
    Wq = np.asarray(inputs["Wq"], dtype=np.float32)
    bq = np.asarray(inputs["bq"], dtype=np.float32)
    Wkv = np.asarray(inputs["Wkv"], dtype=np.float32)
    bkv = np.asarray(inputs["bkv"], dtype=np.float32)
    Wo = np.asarray(inputs["Wo"], dtype=np.float32)
    bo = np.asarray(inputs["bo"], dtype=np.float32)

    hidT_bf = np.ascontiguousarray(hidden[0].T).astype(ml_dtypes.bfloat16)  # [H, S]
    cosT, sinT = _rope_tables()
    ones_col = np.ones((128, 1), np.float32)
    ones_row = np.ones((1, 128), np.float32)
    ident = np.eye(128, dtype=np.float32)

    if causal:
        # diagonal strips from the actual mask (chunk 0 is representative —
        # _check_causal guarantees the pattern is uniform along the diagonal)
        maskT = np.stack(
            [np.ascontiguousarray(mask[0, 0, 0:CH, 128 * j : 128 * j + 128].T)
             for j in range(4)]
        ).astype(ml_dtypes.bfloat16)  # [4, 128, CH]
    else:
        maskT = np.ascontiguousarray(mask[0, 0].T)  # [S, S]

    in_maps = []
    for c in range(N_CORES):
        qs = slice(c * QD, (c + 1) * QD)
        kvs = slice(c * KVD, (c + 1) * KVD)
        in_maps.append(
            {
                "hidT": hidT_bf,
                "wq_t": np.ascontiguousarray(Wq[qs, :].T).astype(ml_dtypes.bfloat16),
                "wkv_t": np.ascontiguousarray(Wkv[kvs, :].T).astype(ml_dtypes.bfloat16),
                "wo_t": np.ascontiguousarray(Wo[qs, :].T).astype(ml_dtypes.bfloat16),
                "bq": np.ascontiguousarray(bq[qs].reshape(NH_LOC, 128).T),
                "bkv": np.ascontiguousarray(
                    bkv[kvs].reshape(2, 128).T
                ),
                "bo": bo[qs].reshape(1, QD),
                "cosT": cosT,
                "sinT": sinT,
                "ones_col": ones_col,
                "ones_row": ones_row,
                "ident": ident,
                "maskT": maskT,
            }
        )
    return in_maps


def kernel(**inputs) -> np.ndarray:
    causal = _check_causal(np.asarray(inputs["attention_mask"], dtype=np.float32))
    nc = _get_nc(causal)
    in_maps = _make_in_maps(inputs, causal)
    res = run_bass_kernel_spmd(nc, in_maps, list(range(N_CORES)))
    outs = [res.results[c]["out"] for c in range(N_CORES)]  # each [S, QD]
    full = np.concatenate(outs, axis=1)  # [S, H]
    return full.reshape(B, S, H)



# revision 21
# speedup vs baseline: 1.0552x; 1.0111x over previous
"""Tensor-parallel GQA attention (CustomLlamaAttention) on 8 TRN2 NeuronCores.

Sharding: heads.  Core c owns Q heads 4c..4c+3 and KV head c.
  - Wq/Wkv output dims sharded; attention fully head-local per core.
  - Output projection sharded over Wo *rows* (output dim): each core computes
    out[:, 512c:512c+512] after an AllGather of the per-core attention
    outputs (transposed layout [hd, s]) -- cheaper than the all-reduce
    variant (4MB gather vs 32MB reduce).

Per-core dataflow (f32 storage, float32r matmuls at full PE rate):
  hidT [4096,2048] -> Q/KV projections -> qT/kT [hd,s] + partial RoPE,
  vT -> PE-transpose -> v [s,hd];  attention with S^T [sk,sq] layout:
  exp on ACT (no max subtraction: scores are O(10), fp32 exp is safe),
  softmax denominators via ones-column matmul, normalization via
  PE-broadcast of reciprocal sums; AllGather of attnT [512,2048] ->
  [4096,2048]; output projection -> out [2048,512] natural layout.

Matmul/DMA instructions can carry only one semaphore wait on this
toolchain (single EVENTS slot in the ISA); waitfix splits excess waits
onto sequencer NOPs.
"""

import sys

sys.path.insert(0, "/opt/trn_rl_repo")

import numpy as np

import concourse.bass as bass
import concourse.mybir as mybir
import concourse.tile as tile
from concourse.bass_utils import run_bass_kernel_spmd

# ---- problem constants (hardcoded per contract) ----
B, S, H = 1, 2048, 4096
NH, NKV, HD = 32, 8, 128
ROT = 64
BASE = 10000.0
N_CORES = 8
NH_LOC = NH // N_CORES  # 4 q heads per core
QD = NH_LOC * HD  # 512 local q dims
KVD = 2 * HD  # 256 local kv dims
CH = 512  # seq chunk (psum bank width in f32)
NCH = S // CH  # 4
ND = H // 128  # 32 contraction tiles
NT = S // 128  # 16 sk tiles
SCALE = 1.0 / float(np.sqrt(HD))

F32 = mybir.dt.float32
F32R = mybir.dt.float32r
BF16 = mybir.dt.bfloat16
AF = mybir.ActivationFunctionType
ALU = mybir.AluOpType


# --------------------------------------------------------------------------
# waitfix: split >1 semaphore waits per instruction onto sequencer NOPs
# --------------------------------------------------------------------------
def _split_waits(nc, max_waits=1):
    from concourse import bass_isa

    isa = nc.isa
    op = isa.Opcode.NEURON_ISA_TPB_OPCODE_NOP
    n_fixed = 0
    for f in nc.m.functions:
        for blk in f.blocks:
            il = blk.instructions
            fixes = []
            for i, inst in enumerate(il):
                si = inst.sync_info
                if si is None or len(si.on_wait) <= max_waits:
                    continue
                fixes.append((i, inst))
            for i, inst in reversed(fixes):
                si = inst.sync_info
                waits = list(si.on_wait)
                keep = waits[-max_waits:]
                nops = []
                for w in waits[:-max_waits]:
                    instr, fixups = bass_isa.isa_struct(isa, op, {})
                    nop = mybir.InstISA(
                        name=nc.get_next_instruction_name(),
                        isa_opcode=op.value,
                        engine=inst.engine,
                        instr=instr,
                        op_name="NOP",
                        ins=[],
                        outs=[],
                        ant_dict={},
                        verify=True,
                        ant_isa_is_sequencer_only=True,
                        ant_sbuf_fixups=fixups or None,
                    )
                    nop.sync_info = mybir.SyncInfo(on_wait=[w], on_update=[])
                    nops.append(nop)
                inst.sync_info = mybir.SyncInfo(on_wait=keep, on_update=si.on_update)
                for j, nop in enumerate(nops):
                    il.insert(i + j, nop)
                n_fixed += 1
    return n_fixed


# --------------------------------------------------------------------------
# kernel builder (SPMD program, same for all 8 cores)
# --------------------------------------------------------------------------
def build(causal: bool, skip_collective: bool = False, skip_waitfix: bool = False) -> bass.Bass:
    nc = bass.Bass()

    hidT = nc.declare_dram_parameter("hidT", [H, S], BF16, isOutput=False)
    wq_t = nc.declare_dram_parameter("wq_t", [H, QD], BF16, isOutput=False)
    wkv_t = nc.declare_dram_parameter("wkv_t", [H, KVD], BF16, isOutput=False)
    wo_t = nc.declare_dram_parameter("wo_t", [H, QD], BF16, isOutput=False)
    bq = nc.declare_dram_parameter("bq", [128, NH_LOC], F32, isOutput=False)
    bkv = nc.declare_dram_parameter("bkv", [128, 2], F32, isOutput=False)
    bo = nc.declare_dram_parameter("bo", [1, QD], F32R, isOutput=False)
    cosT = nc.declare_dram_parameter("cosT", [ROT, S], mybir.dt.bfloat16, isOutput=False)
    sinT = nc.declare_dram_parameter("sinT", [ROT, S], mybir.dt.bfloat16, isOutput=False)
    ones_col = nc.declare_dram_parameter("ones_col", [128, 1], F32R, isOutput=False)
    ones_row = nc.declare_dram_parameter("ones_row", [1, 128], F32R, isOutput=False)
    ident = nc.declare_dram_parameter("ident", [128, 128], F32R, isOutput=False)
    if causal:
        # 4 diagonal mask strips: strip j is the [sk 128, sq 512] transposed
        # mask block whose diagonal offset is 128*j (bf16: 0 / -3.39e38 exact)
        maskT = nc.declare_dram_parameter("maskT", [4, 128, CH], BF16, isOutput=False)
    else:
        maskT = nc.declare_dram_parameter("maskT", [S, S], F32, isOutput=False)
    out = nc.declare_dram_parameter("out", [S, QD], F32, isOutput=True)

    # per-seq-chunk collective tensors: AG(ch) fires as soon as attention
    # for chunk ch is done on all cores, overlapping later attention chunks
    ag_in = [nc.dram_tensor(f"ag_in{ch}", [QD, CH], BF16) for ch in range(NCH)]
    ag_out = [
        nc.dram_tensor(f"ag_out{ch}", [H, CH], BF16, addr_space="Shared")
        for ch in range(NCH)
    ]

    with tile.TileContext(nc) as tc:
        with tc.tile_pool(name="consts", bufs=1) as consts:
            # tiny consts up front; the big tables are DMA'd after chunk 0's
            # first d-group so they don't delay the first matmuls
            ones_col_t = consts.tile([128, 1], F32R)
            nc.sync.dma_start(out=ones_col_t, in_=ones_col[:, :])
            ones_row_t = consts.tile([1, 128], F32R)
            nc.sync.dma_start(out=ones_row_t, in_=ones_row[:, :])
            bq_t = consts.tile([128, NH_LOC], F32)
            nc.sync.dma_start(out=bq_t, in_=bq[:, :])
            bkv_t = consts.tile([128, 2], F32)
            nc.sync.dma_start(out=bkv_t, in_=bkv[:, :])
            ident_t = consts.tile([128, 128], F32R)
            cos_t = consts.tile([ROT, S], mybir.dt.bfloat16)
            sin_t = consts.tile([ROT, S], mybir.dt.bfloat16)
            bo_t = consts.tile([1, QD], F32R)
            mask_t = consts.tile([128, 4, CH], BF16, name="mask_t") if causal else None
            # preload the ACT exp table (~1.3us) far ahead of the first real
            # exp so it is off the phase-A -> attention critical path
            warm_t = consts.tile([1, 1], F32)
            nc.scalar.activation(warm_t[:, :], ones_col_t[0:1, 0:1], AF.Exp)

            # chunk 0 is DMA-throughput-bound (hid + all weights), so only
            # the tables chunk 0's own stores need (cos/sin/ident, ~2us) load
            # there; mask/bo wait for chunk 1 where DMA has ~30us of slack
            def _load_tables():
                nc.sync.dma_start(out=cos_t, in_=cosT[:, :])
                nc.sync.dma_start(out=sin_t, in_=sinT[:, :])
                nc.sync.dma_start(out=ident_t, in_=ident[:, :])

            def _load_mask():
                nc.sync.dma_start(out=bo_t, in_=bo[:, :])
                if causal:
                    nc.sync.dma_start(
                        out=mask_t, in_=maskT.rearrange("j p m -> p j m")
                    )

            # ropetmp/vtmp stay open for the whole kernel: their SBUF ranges
            # must NOT be recycled into attention's pools, or attention's
            # first tiles inherit a WAR on the last rope chain and the
            # whole A->B seam serializes behind it
            with (
                tc.tile_pool(name="qkv", bufs=1) as qkv,
                tc.tile_pool(name="ropetmp", bufs=2) as ropetmp,
                tc.tile_pool(name="vtmp", bufs=1) as vtmp,
            ):
                qT = qkv.tile([128, NH_LOC, S], F32R)  # [hd, head, sq]
                kT = qkv.tile([128, S], F32R)  # [hd, sk]
                vN = qkv.tile([128, NT, HD], F32R)  # [sk%128, sk tile, hd]

                # ---------------- phase A: projections -------------------
                with (
                    tc.tile_pool(name="wqkv", bufs=1) as wpool,
                    tc.tile_pool(name="hid", bufs=16) as hidp,
                    tc.tile_pool(name="ppsum", bufs=1, space="PSUM") as ppsum,
                    tc.tile_pool(name="tppsum", bufs=2, space="PSUM") as tppsum,
                ):
                    # weight tiles are loaded per-d, interleaved with chunk 0's
                    # hid tiles (inside the loop below) so the first matmuls
                    # start after ~2 small DMAs instead of the full 12MB
                    wq_tile = wpool.tile([128, ND, QD], BF16)
                    wkv_tile = wpool.tile([128, ND, KVD], BF16)
                    vT = vtmp.tile([128, S], F32R)  # [hd, sk], pre-transpose

                    for ch in range(NCH):
                        cs = slice(ch * CH, (ch + 1) * CH)
                        psums = [ppsum.tile([128, CH], F32, tag=f"pp{m}", name=f"pp{m}") for m in range(6)]
                        DG = 8
                        NGRP = ND // DG

                        for grp in range(NGRP):
                            hts = []
                            for dl in range(DG):
                                d = grp * DG + dl
                                ht = hidp.tile([128, CH], BF16, tag="hid")
                                nc.sync.dma_start(
                                    out=ht, in_=hidT[d * 128 : (d + 1) * 128, cs]
                                )
                                hts.append(ht)
                                if ch == 0:
                                    nc.sync.dma_start(
                                        out=wq_tile[:, d, :],
                                        in_=wq_t[d * 128 : (d + 1) * 128, :],
                                    )
                                    nc.sync.dma_start(
                                        out=wkv_tile[:, d, :],
                                        in_=wkv_t[d * 128 : (d + 1) * 128, :],
                                    )
                            # const loads queue AFTER the group's hid/weight
                            # DMAs so they never delay the tiles PE is about
                            # to consume; still in time for chunk-0's rope
                            # stores (cos/sin/ident) and attention (mask)
                            if ch == 0 and grp == 2:
                                _load_tables()
                            elif ch == 1 and grp == 1:
                                _load_mask()
                            # d-major: each d consumes its (ht, wq, wkv) DMAs
                            # right after they land, so PE paces the DMA
                            # stream instead of stalling a whole group on it
                            # (chunk 0 is DMA-fed at ~90% of PE rate)
                            for dl in range(DG):
                                d = grp * DG + dl
                                for m in range(6):
                                    if m < NH_LOC:
                                        w_ap = wq_tile[:, :, m * 128 : (m + 1) * 128]
                                    else:
                                        mm = m - NH_LOC
                                        w_ap = wkv_tile[:, :, mm * 128 : (mm + 1) * 128]
                                    nc.tensor.matmul(
                                        psums[m][:, :],
                                        w_ap[:, d, :],
                                        hts[dl][:, :],
                                        start=(d == 0),
                                        stop=(d == ND - 1),
                                        skip_group_check=True,
                                    )
                        # stores: q0 first (frees pp0 for the next chunk's
                        # first matmul), then k (attention needs kT), q1-3, v
                        _rope_store(
                            nc, ropetmp, qT[:, 0, cs], psums[0],
                            bq_t[:, 0:1], cos_t[:, cs], sin_t[:, cs],
                        )
                        _rope_store(
                            nc, ropetmp, kT[:, cs], psums[4],
                            bkv_t[:, 0:1], cos_t[:, cs], sin_t[:, cs],
                        )
                        for m in range(1, NH_LOC):
                            _rope_store(
                                nc, ropetmp, qT[:, m, cs], psums[m],
                                bq_t[:, m : m + 1], cos_t[:, cs], sin_t[:, cs],
                            )
                        nc.vector.tensor_scalar_add(vT[:, cs], psums[5], bkv_t[:, 1:2])

                        # transpose this chunk's v: vT [hd, sk] -> vN [sk, hd]
                        for tl in range(CH // 128):
                            t = ch * (CH // 128) + tl
                            pst = tppsum.tile([128, 128], F32R)
                            nc.tensor.transpose(
                                pst[:, :], vT[:, t * 128 : (t + 1) * 128], ident_t[:, :]
                            )
                            nc.vector.tensor_copy(vN[:, t, :], pst[:, :])

                # phase B + C SBUF pools open together: the wo_t load streams
                # during attention, hiding the 8MB transfer
                with (
                    tc.tile_pool(name="wo", bufs=1) as wop,
                    tc.tile_pool(name="strip", bufs=6) as stripp,
                    tc.tile_pool(name="outp", bufs=3) as outp,
                ):
                    wo_tile = wop.tile([128, ND, QD], BF16)
                    for d in range(ND):
                        nc.sync.dma_start(
                            out=wo_tile[:, d, :],
                            in_=wo_t[d * 128 : (d + 1) * 128, :],
                        )

                    # ---------------- phase B: attention ---------------------
                    with (
                        tc.tile_pool(name="spsum", bufs=3, space="PSUM") as spsum,
                        tc.tile_pool(name="opsum", bufs=2, space="PSUM") as opsum,
                        tc.tile_pool(name="smpsum", bufs=2, space="PSUM") as smpsum,
                        tc.tile_pool(name="bcpsum", bufs=1, space="PSUM") as bcpsum,
                        tc.tile_pool(name="ptile", bufs=10) as ptile,
                        tc.tile_pool(name="btmp", bufs=3) as btmp,
                    ):
                        # start with chunk 2: its q/k tiles have been ready
                        # since A's chunk-2 finished (~44us of slack) and 8
                        # of its 12 sk-tiles are mask-free, so attention
                        # opens at full rate while chunk 3's rope stores
                        # drain on DVE/Pool behind it
                        ch_order = [2, 3, 1, 0] if causal else list(range(NCH))

                        def _fire_ag(ch):
                            # chunked all-gather: fires as soon as chunk ch's
                            # 4 heads are stored; overlaps the remaining
                            # attention chunks and phase C
                            if not skip_collective:
                                nc.gpsimd.collective_compute(
                                    "AllGather",
                                    ALU.bypass,
                                    ins=[ag_in[ch][:, :]],
                                    outs=[ag_out[ch][:, :]],
                                    replica_groups=[list(range(N_CORES))],
                                )

                        def _s2(ent):
                            # deferred diag AV/sum matmuls + normalization for
                            # a head; runs one head behind the score stream so
                            # the mask->exp chains of the diag tiles hide
                            # under the next head's score matmuls
                            ch, h, ps_o, ps_sum, diag, nfree = ent
                            for i, (t, sq0, p) in enumerate(diag):
                                st = (nfree == 0 and i == 0)
                                sp = (i == len(diag) - 1)
                                nc.tensor.matmul(
                                    ps_o[:, sq0:], vN[:, t, :], p[:, sq0:],
                                    start=st, stop=sp, skip_group_check=True,
                                )
                                nc.tensor.matmul(
                                    ps_sum[:, sq0:], ones_col_t[:, :], p[:, sq0:],
                                    start=st, stop=sp, skip_group_check=True,
                                )
                            # normalize: out *= 1/sums (broadcast over parts)
                            rec = btmp.tile([1, CH], F32R, tag="rec")
                            with nc.allow_low_precision(reason="f32r recip"):
                                nc.vector.reciprocal(rec[:, :], ps_sum[:, :])
                            ps_bc = bcpsum.tile([128, CH], F32, tag="bc")
                            nc.tensor.matmul(
                                ps_bc[:, :], ones_row_t[:, :], rec[:, :],
                                start=True, stop=True,
                            )
                            bc_sb = btmp.tile([128, CH], F32, tag="bcs")
                            nc.vector.tensor_copy(bc_sb[:, :], ps_bc[:, :])
                            an = btmp.tile([128, CH], BF16, tag="an")
                            nc.vector.tensor_mul(an[:, :], ps_o[:, :], bc_sb[:, :])
                            nc.sync.dma_start(
                                out=ag_in[ch][h * 128 : (h + 1) * 128, :],
                                in_=an[:, :],
                            )
                            if h == NH_LOC - 1:
                                _fire_ag(ch)

                        if causal:
                            pending = None
                            for ch in ch_order:
                                nt = 4 * (ch + 1)
                                nfree = 4 * ch
                                for h in range(NH_LOC):
                                    # ---- S1: scores + exp; free-tile AV/sum
                                    # lag one tile behind their score ----
                                    ps_o = opsum.tile([128, CH], F32, tag="o")
                                    ps_sum = smpsum.tile([1, CH], F32, tag="s")
                                    diag = []
                                    prev = None  # (t, p) of previous free tile
                                    for t in range(nt):
                                        diag_j = t - 4 * ch
                                        # causal: diag strip j only needs
                                        # sq >= 128j. Cap at 256 so the f32r
                                        # moving dim stays >=256 (full PE
                                        # rate); the mask covers the rest.
                                        sq0 = (
                                            min(128 * diag_j, 256)
                                            if diag_j > 0
                                            else 0
                                        )
                                        qs_ap = qT[:, h, ch * CH + sq0 : (ch + 1) * CH]
                                        ps_s = spsum.tile([128, CH], F32, tag="sc")
                                        nc.tensor.matmul(
                                            ps_s[:, sq0:],
                                            kT[:, t * 128 : (t + 1) * 128],
                                            qs_ap,
                                            start=True,
                                            stop=True,
                                        )
                                        p = ptile.tile([128, CH], F32R, tag="p")
                                        if diag_j >= 0:
                                            # DVE, not Pool: GPSIMD cannot
                                            # read PSUM per the BIR verifier
                                            sb_s = btmp.tile([128, CH], F32, tag="sb")
                                            nc.vector.scalar_tensor_tensor(
                                                sb_s[:, sq0:], ps_s[:, sq0:], SCALE,
                                                mask_t[:, diag_j, sq0:],
                                                op0=ALU.mult, op1=ALU.add,
                                            )
                                            nc.scalar.activation(
                                                p[:, sq0:], sb_s[:, sq0:], AF.Exp
                                            )
                                            diag.append((t, sq0, p))
                                        else:
                                            nc.scalar.activation(
                                                p[:, :], ps_s[:, :], AF.Exp, scale=SCALE
                                            )
                                            if prev is not None:
                                                pt, pp = prev
                                                nc.tensor.matmul(
                                                    ps_o[:, :], vN[:, pt, :], pp[:, :],
                                                    start=(pt == 0), stop=False,
                                                    skip_group_check=True,
                                                )
                                                nc.tensor.matmul(
                                                    ps_sum[:, :], ones_col_t[:, :], pp[:, :],
                                                    start=(pt == 0), stop=False,
                                                    skip_group_check=True,
                                                )
                                            prev = (t, p)
                                    if prev is not None:
                                        pt, pp = prev
                                        nc.tensor.matmul(
                                            ps_o[:, :], vN[:, pt, :], pp[:, :],
                                            start=(pt == 0), stop=False,
                                            skip_group_check=True,
                                        )
                                        nc.tensor.matmul(
                                            ps_sum[:, :], ones_col_t[:, :], pp[:, :],
                                            start=(pt == 0), stop=False,
                                            skip_group_check=True,
                                        )
                                    if pending is not None:
                                        _s2(pending)
                                    pending = (ch, h, ps_o, ps_sum, diag, nfree)
                            _s2(pending)
                        else:
                            for ch in ch_order:
                                cs = slice(ch * CH, (ch + 1) * CH)
                                for h in range(NH_LOC):
                                    ps_o = opsum.tile([128, CH], F32, tag="o")
                                    ps_sum = smpsum.tile([1, CH], F32, tag="s")
                                    for t in range(NT):
                                        qs_ap = qT[:, h, cs]
                                        ps_s = spsum.tile([128, CH], F32, tag="sc")
                                        nc.tensor.matmul(
                                            ps_s[:, :],
                                            kT[:, t * 128 : (t + 1) * 128],
                                            qs_ap,
                                            start=True,
                                            stop=True,
                                        )
                                        mt = btmp.tile([128, CH], F32, tag="mt", bufs=2)
                                        nc.sync.dma_start(
                                            out=mt,
                                            in_=maskT[t * 128 : (t + 1) * 128, cs],
                                        )
                                        sb_s = btmp.tile([128, CH], F32, tag="sb")
                                        nc.vector.scalar_tensor_tensor(
                                            sb_s[:, :], ps_s[:, :], SCALE,
                                            mt[:, :],
                                            op0=ALU.mult, op1=ALU.add,
                                        )
                                        p = ptile.tile([128, CH], F32R, tag="p")
                                        nc.scalar.activation(
                                            p[:, :], sb_s[:, :], AF.Exp
                                        )
                                        nc.tensor.matmul(
                                            ps_o[:, :], vN[:, t, :], p[:, :],
                                            start=(t == 0), stop=(t == NT - 1),
                                            skip_group_check=True,
                                        )
                                        nc.tensor.matmul(
                                            ps_sum[:, :], ones_col_t[:, :], p[:, :],
                                            start=(t == 0), stop=(t == NT - 1),
                                            skip_group_check=True,
                                        )
                                    _s2((ch, h, ps_o, ps_sum, [], 1))

                    # ---------------- phase C: output projection ----------
                    with tc.tile_pool(name="copsum", bufs=2, space="PSUM") as copsum:
                        ps_bo = copsum.tile([128, QD], F32, tag="co0", name="psbo")
                        nc.tensor.matmul(
                            ps_bo[:, :], ones_row_t[:, :], bo_t[:, :],
                            start=True, stop=True,
                        )
                        bo_bc = outp.tile([128, QD], F32, tag="bo")
                        nc.vector.tensor_copy(bo_bc[:, :], ps_bo[:, :])

                        # same order as attention emitted its chunks: sqb's
                        # gather lands earliest for the chunk attention
                        # finished first, so phase C never waits on an AG
                        sq_order = [2, 3, 1, 0] if causal else list(range(NCH))
                        for sqb in sq_order:
                            ps_outs = [
                                copsum.tile(
                                    [128, QD], F32, tag=f"co{j}", name=f"co{j}"
                                )
                                for j in range(4)
                            ]
                            for d in range(ND):
                                strip = stripp.tile([128, CH], BF16, tag="strip")
                                nc.sync.dma_start(
                                    out=strip,
                                    in_=ag_out[sqb][d * 128 : (d + 1) * 128, :],
                                )
                                for j in range(4):
                                    nc.tensor.matmul(
                                        ps_outs[j][:, :],
                                        strip[:, j * 128 : (j + 1) * 128],
                                        wo_tile[:, d, :],
                                        start=(d == 0),
                                        stop=(d == ND - 1),
                                        skip_group_check=True,
                                    )
                            for j in range(4):
                                ot = outp.tile([128, QD], F32, tag="ot")
                                nc.vector.tensor_add(
                                    ot[:, :], ps_outs[j][:, :], bo_bc[:, :]
                                )
                                nc.sync.dma_start(
                                    out=out[
                                        sqb * CH + j * 128 : sqb * CH + (j + 1) * 128,
                                        :,
                                    ],
                                    in_=ot[:, :],
                                )

    if not skip_waitfix:
        _split_waits(nc)
    return nc


def _rope_store(nc, tmp, dst, ps, bias, cos_s, sin_s):
    """dst[hd, s] (f32r) = rope(ps + bias) for rows 0:64; pass rows 64:128.

    cos_s [64, CH]: cos table duplicated over both 32-row halves.
    sin_s [64, CH]: sign-folded sin: rows 0:32 = -sin, rows 32:64 = +sin, so
      new[0:64] = x[0:64]*cos_s + rot32(x[0:64])*sin_s
    with rot32 = swap of the two 32-row halves (done via SBUF->SBUF DMA,
    since compute engines cannot cross partitions).
    """
    R2 = ROT // 2
    # bias-added copy of the whole tile straight into the destination
    nc.vector.tensor_scalar_add(dst[:, :], ps[:, :], bias[:, :])
    # partition-rotated rope rows
    qsh = tmp.tile([ROT, CH], F32R, tag="r1")
    nc.sync.dma_start(out=qsh[0:R2, :], in_=dst[R2:ROT, :])
    nc.sync.dma_start(out=qsh[R2:ROT, :], in_=dst[0:R2, :])
    # the two muls run on the (otherwise idle) Pool engine so the DVE only
    # carries the add chain -- keeps rope off the chunk-boundary critical path
    t1 = tmp.tile([ROT, CH], F32, tag="r2")
    nc.gpsimd.tensor_mul(t1[:, :], dst[0:ROT, :], cos_s[:, :])
    t2 = tmp.tile([ROT, CH], F32, tag="r3")
    nc.gpsimd.tensor_mul(t2[:, :], qsh[:, :], sin_s[:, :])
    nc.vector.tensor_add(dst[0:ROT, :], t1[:, :], t2[:, :])


# --------------------------------------------------------------------------
# host side: shard, run, gather
# --------------------------------------------------------------------------
_NC_CACHE = {}


def _get_nc(causal: bool) -> bass.Bass:
    if causal not in _NC_CACHE:
        _NC_CACHE[causal] = build(causal)
    return _NC_CACHE[causal]


def _rope_tables():
    inv_freq = 1.0 / (BASE ** (np.arange(0, ROT, 2, dtype=np.float64) / ROT))
    t = np.arange(S, dtype=np.float64)
    freqs = np.outer(t, inv_freq)  # [S, 32]
    import ml_dtypes

    cos32 = np.cos(freqs).T.astype(np.float32)  # [32, S]
    sin32 = np.sin(freqs).T.astype(np.float32)
    cosT = np.concatenate([cos32, cos32], axis=0).astype(ml_dtypes.bfloat16)
    sinT = np.concatenate([-sin32, sin32], axis=0).astype(ml_dtypes.bfloat16)
    return cosT, sinT


def _check_causal(mask):
    """mask: [1,1,S,S]. True if it is exactly a causal additive mask."""
    m = mask[0, 0]
    if not (m[np.tril_indices(S)] == 0.0).all():
        return False
    iu = np.triu_indices(S, k=1)
    vals = m[iu]
    return bool((vals <= -1e30).all()) and bool((vals == vals[0]).all())


def _make_in_maps(inputs, causal):
    import ml_dtypes
    hidden = np.asarray(inputs["hidden_states"], dtype=np.float32)
    mask = np.asarray(inputs["attention_mask"], dtype=np.float32)
    Wq = np.asarray(inputs["Wq"], dtype=np.float32)
    bq = np.asarray(inputs["bq"], dtype=np.float32)
    Wkv = np.asarray(inputs["Wkv"], dtype=np.float32)
    bkv = np.asarray(inputs["bkv"], dtype=np.float32)
    Wo = np.asarray(inputs["Wo"], dtype=np.float32)
    bo = np.asarray(inputs["bo"], dtype=np.float32)

    hidT_bf = np.ascontiguousarray(hidden[0].T).astype(ml_dtypes.bfloat16)  # [H, S]
    cosT, sinT = _rope_tables()
    ones_col = np.ones((128, 1), np.float32)
    ones_row = np.ones((1, 128), np.float32)
    ident = np.eye(128, dtype=np.float32)

    if causal:
        # diagonal strips from the actual mask (chunk 0 is representative —
        # _check_causal guarantees the pattern is uniform along the diagonal)
        maskT = np.stack(
            [np.ascontiguousarray(mask[0, 0, 0:CH, 128 * j : 128 * j + 128].T)
             for j in range(4)]
        ).astype(ml_dtypes.bfloat16)  # [4, 128, CH]
    else:
        maskT = np.ascontiguousarray(mask[0, 0].T)  # [S, S]

    in_maps = []
    for c in range(N_CORES):
        qs = slice(c * QD, (c + 1) * QD)
        kvs = slice(c * KVD, (c + 1) * KVD)
        in_maps.append(
            {
                "hidT": hidT_bf,
                "wq_t": np.ascontiguousarray(Wq[qs, :].T).astype(ml_dtypes.bfloat16),
                "wkv_t": np.ascontiguousarray(Wkv[kvs, :].T).astype(ml_dtypes.bfloat16),
                "wo_t": np.ascontiguousarray(Wo[qs, :].T).astype(ml_dtypes.bfloat16),
                "bq": np.ascontiguousarray(bq[qs].reshape(NH_LOC, 128).T),
                "bkv": np.ascontiguousarray(
                    bkv[kvs].reshape(2, 128).T
                ),
                "bo": bo[qs].reshape(1, QD),
                "cosT": cosT,
                "sinT": sinT,
                "ones_col": ones_col,
                "ones_row": ones_row,
                "ident": ident,
                "maskT": maskT,
            }
        )
    return in_maps


def kernel(**inputs) -> np.ndarray:
    causal = _check_causal(np.asarray(inputs["attention_mask"], dtype=np.float32))
    nc = _get_nc(causal)
    in_maps = _make_in_maps(inputs, causal)
    res = run_bass_kernel_spmd(nc, in_maps, list(range(N_CORES)))
    outs = [res.results[c]["out"] for c in range(N_CORES)]  # each [S, QD]
    full = np.concatenate(outs, axis=1)  # [S, H]
    return full.reshape(B, S, H)

